# revision 18
# baseline (speedup 1.0000x reference)
"""Trainium2 Bass kernel for nn_DMS_STGAT (dual-branch GAT attention softmaxes).

Strategy (per core, data-parallel over batch B=16 -> 2 per core):
  The reference only uses h = x @ W through two dots s1 = h@a[:F], s2 = h@a[F:],
  so  e[bt, n1, n2] = LRelu(s1[r1[m]] + s2[r2[m]])  with fixed index maps r1/r2
  (the "scrambled pairing").  We compute s1/s2 as 128-dim dots with
  host-precomputed Wa = W@a vectors, gather via tiny host-precomputed 0/1
  matrices Q1/Q2 using PE matmuls, then run the double softmax on-chip.
  Spatial PE term exp(-||x_j - x_8||/1000) enters as a third accumulated
  matmul with Qs = S1*Q1 + S2*Q2.  Temporal positional term is a constant row
  qadj (pre-shifted per softmax group so exp never overflows).
"""
import sys
import numpy as np

for _p in ("/opt/trn_rl_repo", "/root/.axon_site/_ro/trn_rl_repo"):
    if _p not in sys.path:
        sys.path.insert(0, _p)

from contextlib import ExitStack  # noqa: E402

import concourse.bass as bass  # noqa: E402
import concourse.tile as tile  # noqa: E402
from concourse import bacc, mybir  # noqa: E402

B, C, T, J, F = 16, 128, 25, 25, 256
N = 25            # N == T == J
NN = N * N        # 625
NCORES = 8
BL = B // NCORES  # 2 batches per core
FP = mybir.dt.float32
AF = mybir.ActivationFunctionType
ALU = mybir.AluOpType

# ---------------------------------------------------------------- host math --

def _pair_indices():
    """r1[m], r2[m] for m = n1*N + n2 (original flat order)."""
    r1 = np.zeros(NN, np.int64)
    r2 = np.zeros(NN, np.int64)
    for m in range(NN):
        k1, k2 = 2 * m, 2 * m + 1
        r1[m] = (k1 // N) if k1 < NN else ((k1 - NN) % N)
        r2[m] = (k2 // N) if k2 < NN else ((k2 - NN) % N)
    return r1, r2


def _sinusoid_pos():
    pos = np.arange(200)[:, None].astype(np.float64)
    hid = np.arange(C)[None, :]
    angle = pos / np.power(10000.0, 2.0 * (hid // 2) / C)
    tab = angle.copy()
    tab[:, 0::2] = np.sin(angle[:, 0::2])
    tab[:, 1::2] = np.cos(angle[:, 1::2])
    return tab[:T] * 1000.0  # [T, C] float64


_R1, _R2 = _pair_indices()


def _host_consts(W_s, a_s, W_t, a_t):
    """Precompute tiny derived params in float64. ~0.3 MFLOP."""
    W_s = W_s.astype(np.float64)
    a_s = a_s.astype(np.float64)
    W_t = W_t.astype(np.float64)
    a_t = a_t.astype(np.float64)
    wa_s1 = W_s @ a_s[:F, 0]
    wa_s2 = W_s @ a_s[F:, 0]
    wa_t1 = W_t @ a_t[:F, 0]
    wa_t2 = W_t @ a_t[F:, 0]
    S1, S2 = wa_s1.sum(), wa_s2.sum()

    Q1 = np.zeros((N, NN), np.float64)
    Q2 = np.zeros((N, NN), np.float64)
    Q1[_R1, np.arange(NN)] = 1.0
    Q2[_R2, np.arange(NN)] = 1.0

    qs = S1 * Q1 + S2 * Q2                      # [25, 625]

    pos = _sinusoid_pos()                       # [25, 128]
    p1 = pos @ wa_t1
    p2 = pos @ wa_t2
    qp = p1[_R1] + p2[_R2]                      # [625] true temporal constant
    # post-LRelu shift constant, per n2-group (softmax-invariant, exp-safe)
    qLR = np.where(qp > 0, qp, 0.2 * qp)
    cq = qLR.reshape(N, N).max(axis=0)          # max over n1, per n2
    csh = cq[np.arange(NN) % N][None, :]        # [1, 625]

    wa4 = np.stack([wa_s1, wa_s2, wa_t1, wa_t2], axis=1)  # [128, 4]
    q1a = np.concatenate([Q1, qp[None, :]], axis=0)       # [26, 625]
    return (wa4.astype(np.float32), q1a.astype(np.float32),
            Q2.astype(np.float32), qs.astype(np.float32),
            csh.astype(np.float32))


# ------------------------------------------------------------- bass program --

def _build_program():
    nc = bacc.Bacc("TRN2", target_bir_lowering=False, debug=False)

    src_d = nc.dram_tensor("src_l", [BL, C, T, J], FP, kind="ExternalInput").ap()
    wa4_d = nc.dram_tensor("wa4", [C, 4], FP, kind="ExternalInput").ap()
    q1a_d = nc.dram_tensor("q1a", [N + 1, NN], FP, kind="ExternalInput").ap()
    q2_d = nc.dram_tensor("q2", [N, NN], FP, kind="ExternalInput").ap()
    qs_d = nc.dram_tensor("qs", [N, NN], FP, kind="ExternalInput").ap()
    csh_d = nc.dram_tensor("csh", [1, NN], FP, kind="ExternalInput").ap()
    outs_d = nc.dram_tensor("out_s", [BL, T, N, N], FP, kind="ExternalOutput").ap()
    outt_d = nc.dram_tensor("out_t", [BL, T, N, N], FP, kind="ExternalOutput").ap()

    with tile.TileContext(nc) as tc, ExitStack() as ctx:
        consts = ctx.enter_context(tc.tile_pool(name="consts", bufs=1))
        data = ctx.enter_context(tc.tile_pool(name="data", bufs=1))
        pp = ctx.enter_context(tc.tile_pool(name="pp", bufs=1, space="PSUM"))

        # --- constants / warm-up ---
        dummy = consts.tile([1, 2], FP)
        nc.vector.memset(dummy[:], 0.0)
        nc.scalar.activation(dummy[:], dummy[:], AF.Exp)  # pull ACT table load early

        wa4 = consts.tile([C, 4], FP)
        nc.sync.dma_start(wa4[:], wa4_d)
        q1a = consts.tile([N + 1, NN], FP)
        nc.sync.dma_start(q1a[:], q1a_d)
        q2t = consts.tile([N, NN], FP)
        nc.sync.dma_start(q2t[:], q2_d)
        qst = consts.tile([N, NN], FP)
        nc.sync.dma_start(qst[:], qs_d)
        ones = consts.tile([C, 1], FP)
        nc.vector.memset(ones[:], 1.0)
        # temporal post-LRelu shift constant, broadcast to partitions 64:114
        CSHt = consts.tile([114, NN], FP)
        csh_b = bass.AP(tensor=csh_d.tensor, offset=csh_d.offset, ap=[[0, 50], [1, NN]])
        nc.gpsimd.dma_start(CSHt[64:114, :], csh_b)

        # --- input: X [128, (b, t, j)] ---
        X = data.tile([C, BL * NN], FP)
        for b in range(BL):
            src_b = bass.AP(tensor=src_d.tensor, offset=src_d.offset + b * C * NN,
                            ap=[[NN, C], [1, NN]])
            nc.sync.dma_start(X[:, b * NN:(b + 1) * NN], src_b)

        # --- D2 = (X - ref)^2 (spatial PE distance), per b ---
        D = data.tile([C, BL * NN], FP)
        D2 = data.tile([C, BL * NN], FP)
        FX = X[:].ap[0][0]   # X free pitch
        FD = D[:].ap[0][0]
        for b in range(BL):
            in0 = bass.AP(tensor=X.tensor, offset=X.offset + b * NN,
                          ap=[[FX, C], [N, N], [1, N]])
            ref = bass.AP(tensor=X.tensor, offset=X.offset + b * NN + 8,
                          ap=[[FX, C], [N, N], [0, N]])
            dout = bass.AP(tensor=D.tensor, offset=D.offset + b * NN,
                           ap=[[FD, C], [N, N], [1, N]])
            nc.gpsimd.tensor_tensor(dout, in0, ref, op=ALU.subtract)
            # square: b=0 on ACT, b=1 on DVE (engine spread)
            if b == 0:
                nc.scalar.activation(D2[:, b * NN:(b + 1) * NN],
                                     D[:, b * NN:(b + 1) * NN], AF.Square)
            else:
                nc.vector.tensor_tensor(D2[:, b * NN:(b + 1) * NN],
                                        D[:, b * NN:(b + 1) * NN],
                                        D[:, b * NN:(b + 1) * NN], op=ALU.mult)

        # --- dot-product passes (PE), chunked stationary ---
        # E-psum allocated first so its 512-col chunks are bank-aligned
        psum_E = pp.tile([114, 1024], FP)  # 2 full banks; cols 0:625 used
        # spatial: chunks of 125 cols (5 t-blocks); psum_sd [125, 30]:
        #   col (b*5+ck)*3 + {0:s1, 1:s2, 2:d2}
        psum_sd = pp.tile([125, 30], FP)
        for b in range(BL):
            for ck in range(5):
                q = b * 5 + ck
                nc.tensor.matmul(psum_sd[:, q * 3:q * 3 + 2],
                                 X[:, q * 125:(q + 1) * 125], wa4[:, 0:2],
                                 start=True, stop=True)
        for b in range(BL):
            for ck in range(5):
                q = b * 5 + ck
                nc.tensor.matmul(psum_sd[:, q * 3 + 2:q * 3 + 3],
                                 D2[:, q * 125:(q + 1) * 125], ones[:],
                                 start=True, stop=True)
        # temporal: per-j stride-25 stationary; psum_td [50 (b,t), 50 (j,d)]
        psum_td = pp.tile([50, 50], FP)
        for j in range(N):
            lhsT = bass.AP(tensor=X.tensor, offset=X.offset + j,
                           ap=[[FX, C], [N, 50]])   # (c; (b,t) stride-25 run)
            nc.tensor.matmul(psum_td[:, j * 2:j * 2 + 2],
                             lhsT, wa4[:, 2:4], start=True, stop=True)

        # --- PSUM -> SBUF ---
        SD = data.tile([125, 30], FP)
        nc.scalar.copy(SD[:], psum_sd[:])
        TD = data.tile([50, 50], FP)
        nc.vector.tensor_copy(TD[:], psum_td[:])

        # --- rearranges (partition-aligned block DMAs) ---
        # SP[j, (b*25+t)*3 + d] = SD[ts*25+j, (b*5+ck)*3+d],  t = ck*5+ts
        SP = data.tile([N, 50 * 3], FP)
        FSD = SD[:].ap[0][0]
        FSP = SP[:].ap[0][0]
        for ts in range(5):
            src = bass.AP(tensor=SD.tensor, offset=SD.offset + (ts * 25) * FSD,
                          ap=[[FSD, N], [3, 10], [1, 3]])       # (j, bck, d)
            dst = bass.AP(tensor=SP.tensor, offset=SP.offset + ts * 3,
                          ap=[[FSP, N], [15, 10], [1, 3]])      # (j, bck, d)
            nc.sync.dma_start(dst, src)
        # TPa[t, (b*25+j)*2 + d] = TD[b*25+t, j*2+d]; row 25 = ones
        TPa = data.tile([N + 1, 50 * 2], FP)
        FTD = TD[:].ap[0][0]
        FTP = TPa[:].ap[0][0]
        onesrow = consts.tile([1, 50 * 2], FP)
        nc.vector.memset(onesrow[:], 1.0)
        nc.sync.dma_start(TPa[N:N + 1, :], onesrow[:])
        for b in range(BL):
            src = bass.AP(tensor=TD.tensor, offset=TD.offset + (b * 25) * FTD,
                          ap=[[FTD, N], [2, N], [1, 2]])        # (t, j, d)
            dst = bass.AP(tensor=TPa.tensor, offset=TPa.offset + (b * 25) * 2,
                          ap=[[FTP, N], [2, N], [1, 2]])        # (t, j, d)
            nc.sync.dma_start(dst, src)

        # --- EC = exp(-sqrt(d2s)/1000) via exp(0.5*ln) (same ACT table set) ---
        d2view = bass.AP(tensor=SP.tensor, offset=SP.offset + 2, ap=[[FSP, N], [3, 50]])
        eps_b = consts.tile([N, 1], FP)
        nc.vector.memset(eps_b[:], 1e-30)
        ecL = data.tile([N, 50], FP)
        nc.scalar.activation(ecL[:], d2view, AF.Ln, bias=eps_b[:])
        ecW = data.tile([N, 50], FP)
        nc.scalar.activation(ecW[:], ecL[:], AF.Exp, scale=0.5)
        EC = data.tile([N, 50], FP)
        nc.scalar.activation(EC[:], ecW[:], AF.Exp, scale=-0.001)

        # --- E matmuls: psum_E [114, 625]; spatial rows 0:50, temporal 64:114 ---
        nc.vector.memset(psum_E[32:64, 0:NN], 0.0)  # covers junk rows 50:64; 32:50 re-written by PE
        sp_s1 = bass.AP(tensor=SP.tensor, offset=SP.offset + 0, ap=[[FSP, N], [3, 50]])
        sp_s2 = bass.AP(tensor=SP.tensor, offset=SP.offset + 1, ap=[[FSP, N], [3, 50]])
        tp_t1 = bass.AP(tensor=TPa.tensor, offset=TPa.offset + 0, ap=[[FTP, N + 1], [2, 50]])
        tp_t2 = bass.AP(tensor=TPa.tensor, offset=TPa.offset + 1, ap=[[FTP, N], [2, 50]])
        chunks = [(0, 512), (512, NN)]
        for lo, hi in chunks:
            nc.tensor.matmul(psum_E[0:50, lo:hi], sp_s1, q1a[0:N, lo:hi],
                             start=True, stop=False)
        for lo, hi in chunks:
            nc.tensor.matmul(psum_E[0:50, lo:hi], sp_s2, q2t[:, lo:hi],
                             start=False, stop=False)
        for lo, hi in chunks:
            nc.tensor.matmul(psum_E[0:50, lo:hi], EC[:], qst[:, lo:hi],
                             start=False, stop=True)
        for lo, hi in chunks:
            nc.tensor.matmul(psum_E[64:114, lo:hi], tp_t1, q1a[0:N + 1, lo:hi],
                             start=True, stop=False, tile_position=(0, 64))
        for lo, hi in chunks:
            nc.tensor.matmul(psum_E[64:114, lo:hi], tp_t2, q2t[:, lo:hi],
                             start=False, stop=True, tile_position=(0, 64))

        # --- LRelu(0.2): E2 = max(E, 0.2*E) ---
        t0 = data.tile([114, NN], FP)
        nc.scalar.mul(t0[:], psum_E[:, 0:NN], 0.2)
        E2 = data.tile([114, NN], FP)
        nc.vector.tensor_tensor(E2[:], psum_E[:, 0:NN], t0[:], op=ALU.max)
        # temporal rows: subtract per-group shift so exp stays in range
        nc.vector.tensor_tensor(E2[64:114, :], E2[64:114, :], CSHt[64:114, :],
                                op=ALU.subtract)

        # --- softmax over n1 (axis with stride 25), twice ---
        g = data.tile([114, NN], FP)
        nc.scalar.activation(g[:], E2[:], AF.Exp)
        FG = g[:].ap[0][0]
        Z = data.tile([114, N], FP)
        g_red = bass.AP(tensor=g.tensor, offset=g.offset, ap=[[FG, 114], [1, N], [N, N]])
        nc.vector.tensor_reduce(Z[:], g_red, axis=mybir.AxisListType.X, op=ALU.add)
        Zr = data.tile([114, N], FP)
        nc.vector.reciprocal(Zr[:], Z[:])
        att1 = data.tile([114, NN], FP)
        FZ = Zr[:].ap[0][0]
        FA = att1[:].ap[0][0]
        g_3d = bass.AP(tensor=g.tensor, offset=g.offset, ap=[[FG, 114], [N, N], [1, N]])
        zr_b = bass.AP(tensor=Zr.tensor, offset=Zr.offset, ap=[[FZ, 114], [0, N], [1, N]])
        a1_3d = bass.AP(tensor=att1.tensor, offset=att1.offset, ap=[[FA, 114], [N, N], [1, N]])
        nc.vector.tensor_tensor(a1_3d, g_3d, zr_b, op=ALU.mult)

        g2 = data.tile([114, NN], FP)
        nc.scalar.activation(g2[:], att1[:], AF.Exp)
        FG2 = g2[:].ap[0][0]
        Z2 = data.tile([114, N], FP)
        g2_red = bass.AP(tensor=g2.tensor, offset=g2.offset, ap=[[FG2, 114], [1, N], [N, N]])
        nc.vector.tensor_reduce(Z2[:], g2_red, axis=mybir.AxisListType.X, op=ALU.add)
        Z2r = data.tile([114, N], FP)
        nc.vector.reciprocal(Z2r[:], Z2[:])
        outF = data.tile([114, NN], FP)
        FZ2 = Z2r[:].ap[0][0]
        FO = outF[:].ap[0][0]
        g2_3d = bass.AP(tensor=g2.tensor, offset=g2.offset, ap=[[FG2, 114], [N, N], [1, N]])
        z2_b = bass.AP(tensor=Z2r.tensor, offset=Z2r.offset, ap=[[FZ2, 114], [0, N], [1, N]])
        o_3d = bass.AP(tensor=outF.tensor, offset=outF.offset, ap=[[FO, 114], [N, N], [1, N]])
        nc.vector.tensor_tensor(o_3d, g2_3d, z2_b, op=ALU.mult)

        # --- outputs ---
        outs_flat = bass.AP(tensor=outs_d.tensor, offset=outs_d.offset,
                            ap=[[NN, 50], [1, NN]])
        outt_flat = bass.AP(tensor=outt_d.tensor, offset=outt_d.offset,
                            ap=[[NN, 50], [1, NN]])
        nc.sync.dma_start(outs_flat, outF[0:50, :])
        nc.sync.dma_start(outt_flat, outF[64:114, :])

    nc.compile()
    return nc


_PROGRAM = None


def _get_program():
    global _PROGRAM
    if _PROGRAM is None:
        _PROGRAM = _build_program()
    return _PROGRAM


# ------------------------------------------------------------------ kernel --

def kernel(src, W_s, a_s, W_t, a_t):
    from concourse.bass_utils import run_bass_kernel_spmd

    src = np.ascontiguousarray(np.asarray(src, dtype=np.float32))
    wa4, q1a, q2, qs, csh = _host_consts(np.asarray(W_s), np.asarray(a_s),
                                         np.asarray(W_t), np.asarray(a_t))
    nc = _get_program()
    in_maps = []
    for c in range(NCORES):
        in_maps.append({
            "src_l": src[c * BL:(c + 1) * BL],
            "wa4": wa4, "q1a": q1a, "q2": q2, "qs": qs, "csh": csh,
        })
    res = run_bass_kernel_spmd(nc, in_maps, core_ids=list(range(NCORES)))
    out_s = np.concatenate([res.results[c]["out_s"] for c in range(NCORES)], axis=0)
    out_t = np.concatenate([res.results[c]["out_t"] for c in range(NCORES)], axis=0)
    return out_s, out_t
